# revision 7
# baseline (speedup 1.0000x reference)
"""Trainium2 Bass kernel for a 6-layer GPT-style transformer
(B=64, T=256, V=65, D=384, H=6, FF=1536), data-parallel over batch on 8
NeuronCores (8 batch elements = 2048 tokens per core).

Design notes (v2, restructured from the baseline for engine balance):
  - Embedding (tok_emb[idx] + pos) is computed host-side; the kernel DMAs
    the 16 residual tiles [128, 384] f32 directly.
  - Residual x token-major fp32; LayerNorm via bn_stats/bn_aggr + fused
    tensor_scalar, cast bf16 and DMA-transposed into feature-major xnT.
    LN scale/bias folded into downstream weights host-side.
  - qT/kT feature-major (weights-stationary, N=512 streams); v token-major.
  - Attention processed per (batch, head-pair):
      * scores for both heads of a pair live in one PSUM tile
        (even head cols 0:256, odd head cols 256:512); causal masking is
        done by accumulating -30000 * triangle into the diagonal blocks
        with extra matmuls (lhsT = strictly-upper -30000, rhs = identity),
        so no separate element-wise mask op is needed.
      * one Exp per score tile (bf16 out), softmax denominators via
        ones-matmul row sums into a [128,256] l-tile (even head rows 0:64,
        odd head rows 64:128), one DVE reciprocal, and normalization fused
        into the single PSUM->SBUF copy of the pair's attention output.
      * odd head's o accumulates into PSUM partitions 64:127 so both heads
        land in oT with plain engine ops (no SBUF-SBUF DMA).
  - proj/ff2 token-major (activation-stationary); ff1 feature-major.
  - All per-feature biases are zero for this model instance; the fast
    graph (biases=False) omits them. biases=True adds K=1 ones-row bias
    matmuls and ACT bias operands (general path).
  - Issue order software-pipelines chunks: qk/v of chunk n+1 are issued
    before attention of chunk n so the PE always has independent GEMM
    work while ACT/DVE run softmax.
"""

import numpy as np
import ml_dtypes

import concourse.bass as bass
import concourse.mybir as mybir
import concourse.tile as tile
from concourse.bass_utils import run_bass_kernel_spmd
from contextlib import ExitStack

F32 = mybir.dt.float32
BF16 = mybir.dt.bfloat16
AF = mybir.ActivationFunctionType
OP = mybir.AluOpType

B, T, V, D, H, L = 64, 256, 65, 384, 6, 6
HD = D // H          # 64
FF = 4 * D           # 1536
EPS = 1e-5
SCALE = D ** -0.5
NEG = -30000.0

NCORES = 8
BPC = B // NCORES    # 8 batch elements per core
NTOK = BPC * T       # 2048 tokens per core
TT = NTOK // 128     # 16 token tiles
KT = D // 128        # 3 feature tiles
FT = FF // 128       # 12 ff tiles
NCH = 4              # 512-token chunks
HP = H // 2          # 3 head pairs


def _split_multi_waits(nc):
    """This walrus build rejects >1 sync wait per instruction; hoist extras
    onto standalone EventSemaphore instructions on the same engine queue."""
    ctr = 0
    for func in nc.m.functions:
        for bb in func.blocks:
            insts = list(bb.instructions)
            out = []
            changed = False
            for inst in insts:
                si = inst.sync_info
                if si is not None and len(si.on_wait) > 1:
                    waits = list(si.on_wait)
                    for w in waits[:-1]:
                        ev = mybir.InstEventSemaphore(
                            name=f"splitwait_{ctr}", ins=[], outs=[])
                        ctr += 1
                        ev.engine = inst.engine
                        ev.sync_info = mybir.SyncInfo(on_wait=[w], on_update=[])
                        nc.register_instruction(ev, overwrite=True)
                        out.append(ev)
                    inst.sync_info = mybir.SyncInfo(
                        on_wait=[waits[-1]], on_update=list(si.on_update))
                    changed = True
                out.append(inst)
            if changed:
                bb.instructions = out


DBG = None  # stage tag set by tests


def build(n_layers=L, biases=False):
    nc = bass.Bass(trn_type="TRN2", num_devices=NCORES)

    def din(name, shape, dt):
        return nc.dram_tensor(name, shape, dt, kind="ExternalInput").ap()

    x0_in = din("x0", [TT, 128, D], F32)
    triW = din("triW", [128, 128], BF16)       # strictly-upper -30000
    identI2 = din("identI2", [128, 256], BF16)  # [I | I]
    ones64 = din("ones64", [128, 64], BF16)
    if n_layers:
        wqkv = din("wqkv", [n_layers, 128, KT, 3 * D], BF16)
        wproj = din("wproj", [n_layers, 128, KT, D], BF16)
        wff1 = din("wff1", [n_layers, 128, KT, FF], BF16)
        wff2 = din("wff2", [n_layers, 128, FT, D], BF16)
        if biases:
            bqk = din("bqk", [n_layers, 128, 6], F32)
            bproj = din("bproj", [n_layers, 1, D], BF16)
            bff1 = din("bff1", [n_layers, 128, FT], F32)
            bff2 = din("bff2", [n_layers, 1, D], BF16)
    whead = din("whead", [128, KT, V], BF16)
    if biases:
        bhead = din("bhead", [1, V], BF16)
        ones_row = din("ones_row", [1, 128], BF16)
    logits = nc.dram_tensor("logits", [NTOK, V], F32, kind="ExternalOutput").ap()

    dbg_spec = {
        "x0": (TT, 128, D, F32), "xnT": (KT, 128, NTOK, BF16),
        "qT": (KT, 128, NTOK, BF16), "kT": (KT, 128, NTOK, BF16),
        "vt": (TT, 128, D, BF16), "oT": (KT, 128, NTOK, BF16),
        "x1": (TT, 128, D, F32), "x2": (TT, 128, D, F32),
        "e0": (HP, 128, 512, BF16), "linv": (HP, 128, 256, F32),
    }
    dbg_ap = None
    if DBG is not None:
        n, p, c, dt = dbg_spec[DBG]
        dbg_ap = nc.dram_tensor("dbg", [n, p, c], dt, kind="ExternalOutput").ap()

    def dump(tag, tiles):
        if DBG == tag:
            for i, tl in enumerate(tiles):
                nc.sync.dma_start(out=dbg_ap[i], in_=tl)

    def dump3(tag, t3):
        if DBG == tag:
            for i in range(KT):
                nc.sync.dma_start(out=dbg_ap[i], in_=t3[:, i, :])

    with tile.TileContext(nc) as tc, ExitStack() as ctx:
        pool = lambda name, bufs: ctx.enter_context(tc.tile_pool(name=name, bufs=bufs))
        const_p = pool("const", 1)
        xres_p = pool("xres", 1)
        xnT_p = pool("xnT", 1)
        qkT_p = pool("qkT", 1)
        v_p = pool("vtok", 1)
        oT_p = pool("oT", 1)
        h_p = pool("hff", 2)
        w_p = pool("wts", 2)
        ln_p = pool("ln", 4)
        at_p = pool("attn", 3)
        cp_p = pool("cpy", 3)
        big_ps = ctx.enter_context(
            tc.tile_pool(name="big_ps", bufs=2, space="PSUM"))
        sc_ps = ctx.enter_context(
            tc.tile_pool(name="sc_ps", bufs=2, space="PSUM"))
        lo_ps = ctx.enter_context(
            tc.tile_pool(name="lo_ps", bufs=2, space="PSUM"))

        # ---- constants ----
        tri_s = const_p.tile([128, 128], BF16)
        nc.sync.dma_start(out=tri_s, in_=triW)
        id2_s = const_p.tile([128, 256], BF16)
        nc.sync.dma_start(out=id2_s, in_=identI2)
        ones64_s = const_p.tile([128, 64], BF16)
        nc.sync.dma_start(out=ones64_s, in_=ones64)
        whead_s = const_p.tile([128, KT, V], BF16)
        nc.sync.dma_start(out=whead_s, in_=whead)
        eps_t = const_p.tile([128, 1], F32)
        nc.vector.memset(eps_t, EPS)
        if biases:
            bhead_s = const_p.tile([1, V], BF16)
            nc.sync.dma_start(out=bhead_s, in_=bhead)
            ones_s = const_p.tile([1, 128], BF16)
            nc.sync.dma_start(out=ones_s, in_=ones_row)

        # ---- persistent activation tiles ----
        x = [xres_p.tile([128, D], F32, tag=f"x{t}", name=f"x{t}") for t in range(TT)]
        xnT = xnT_p.tile([128, KT, NTOK], BF16, tag="xnT", name="xnT")
        qT = [qkT_p.tile([128, NTOK], BF16, tag=f"qT{k}", name=f"qT{k}") for k in range(KT)]
        kT = [qkT_p.tile([128, NTOK], BF16, tag=f"kT{k}", name=f"kT{k}") for k in range(KT)]
        vt = [v_p.tile([128, D], BF16, tag=f"v{t}", name=f"v{t}") for t in range(TT)]
        oT = [oT_p.tile([128, NTOK], BF16, tag=f"oT{k}", name=f"oT{k}") for k in range(KT)]

        # ---- load x0 ----
        for t in range(TT):
            nc.sync.dma_start(out=x[t], in_=x0_in[t])
        dump("x0", x)

        def ln_tile(t):
            """token-major LN of x[t] -> bf16 -> DMA-transpose into xnT."""
            stats = ln_p.tile([128, 6], F32, tag="stats")
            nc.vector.bn_stats(out=stats, in_=x[t])
            mv = ln_p.tile([128, 2], F32, tag="mv")
            nc.vector.bn_aggr(out=mv, in_=stats)
            rstd = ln_p.tile([128, 1], F32, tag="rstd")
            nc.scalar.activation(out=rstd, in_=mv[:, 1:2], func=AF.Sqrt,
                                 bias=eps_t)
            nc.vector.reciprocal(out=rstd, in_=rstd)
            xn16 = ln_p.tile([128, D], BF16, tag="xn16")
            nc.vector.tensor_scalar(out=xn16, in0=x[t], scalar1=mv[:, 0:1],
                                    scalar2=rstd, op0=OP.subtract, op1=OP.mult)
            nc.sync.dma_start(out=xnT[:, :, t * 128:(t + 1) * 128],
                              in_=xn16, transpose=True)

        def qkv_chunk(n, wqkv_s, bqk_s):
            """qT/kT (feature-major) and v (token-major) for 512-token chunk."""
            ns = slice(n * 512, (n + 1) * 512)
            for m in range(6):
                dst = qT[m] if m < KT else kT[m - KT]
                ps = big_ps.tile([128, 512], F32, tag="big")
                for k in range(KT):
                    nc.tensor.matmul(
                        ps, lhsT=wqkv_s[:, k, m * 128:(m + 1) * 128],
                        rhs=xnT[:, k, ns], start=(k == 0), stop=(k == KT - 1))
                if biases:
                    nc.scalar.activation(out=dst[:, ns], in_=ps, func=AF.Identity,
                                         bias=bqk_s[:, m:m + 1])
                else:
                    nc.scalar.activation(out=dst[:, ns], in_=ps, func=AF.Identity)
            for tt in range(4):
                t = n * 4 + tt
                ps = big_ps.tile([128, 512], F32, tag="big")
                for k in range(KT):
                    nc.tensor.matmul(ps[:, 0:D],
                                     lhsT=xnT[:, k, t * 128:(t + 1) * 128],
                                     rhs=wqkv_s[:, k, 2 * D:3 * D],
                                     start=(k == 0), stop=(k == KT - 1))
                nc.vector.tensor_copy(out=vt[t], in_=ps[:, 0:D])

        def attn_batch(b, l):
            """attention for batch b, all 3 head pairs."""
            n0 = b * T
            sc0s, sc1s = [], []
            for j in range(HP):
                he, ho = 2 * j, 2 * j + 1
                # scores: even head cols 0:512? (q 0:256) / odd cols 256:512
                sc0 = sc_ps.tile([128, 512], F32, tag="sc0")
                sc1 = sc_ps.tile([128, 256], F32, tag="sc1")
                for i, r in ((0, 0), (1, 64)):
                    # first mm into each PSUM bank starts the group; the
                    # second covers fresh (pending-zero) bytes with start=False
                    nc.tensor.matmul(
                        sc0[:, i * 256:(i + 1) * 256],
                        lhsT=kT[j][r:r + 64, n0:n0 + 128],
                        rhs=qT[j][r:r + 64, n0:n0 + 256],
                        start=(i == 0), stop=False)
                    nc.tensor.matmul(
                        sc1[:, i * 128:(i + 1) * 128],
                        lhsT=kT[j][r:r + 64, n0 + 128:n0 + 256],
                        rhs=qT[j][r:r + 64, n0 + 128:n0 + 256],
                        start=(i == 0), stop=False)
                # causal masks on diagonal blocks (q 0:128 of each head in
                # sc0; all of sc1)
                nc.tensor.matmul(sc0[:, 0:128], lhsT=tri_s,
                                 rhs=id2_s[:, 0:128], start=False, stop=False)
                nc.tensor.matmul(sc0[:, 256:384], lhsT=tri_s,
                                 rhs=id2_s[:, 0:128], start=False, stop=True)
                nc.tensor.matmul(sc1, lhsT=tri_s, rhs=id2_s,
                                 start=False, stop=True)
                sc0s.append(sc0)
                sc1s.append(sc1)
            e0s, e1s = [], []
            for j in range(HP):
                e0 = at_p.tile([128, 512], BF16, tag="e0")
                nc.scalar.activation(out=e0, in_=sc0s[j], func=AF.Exp,
                                     scale=SCALE)
                e1 = at_p.tile([128, 256], BF16, tag="e1")
                nc.scalar.activation(out=e1, in_=sc1s[j], func=AF.Exp,
                                     scale=SCALE)
                e0s.append(e0)
                e1s.append(e1)
            if DBG == "e0" and l == 0 and b == 7:
                for j in range(HP):
                    nc.sync.dma_start(out=dbg_ap[j], in_=e0s[j])
            for j in range(HP):
                he, ho = 2 * j, 2 * j + 1
                e0, e1 = e0s[j], e1s[j]
                # One PSUM bank per head: a single accumulation group on a
                # single partition-row range (even rows 0:64, odd 64:128).
                # cols 0:256 = softmax denominators, 256:512 = output.
                loE = lo_ps.tile([128, 512], F32, tag="lo", name="loE")
                loO = lo_ps.tile([128, 512], F32, tag="lo", name="loO")
                l_e, o_e = loE[0:64, 0:256], loE[0:64, 256:512]
                l_o, o_o = loO[64:128, 0:256], loO[64:128, 256:512]
                nc.tensor.matmul(l_e, lhsT=ones64_s,
                                 rhs=e0[:, 0:256], start=True, stop=False)
                nc.tensor.matmul(l_o, lhsT=ones64_s,
                                 rhs=e0[:, 256:512], start=True, stop=False)
                nc.tensor.matmul(l_e[:, 128:256], lhsT=ones64_s,
                                 rhs=e1[:, 0:128], start=False, stop=False)
                nc.tensor.matmul(l_o[:, 128:256], lhsT=ones64_s,
                                 rhs=e1[:, 128:256], start=False, stop=False)
                nc.tensor.matmul(o_e,
                                 lhsT=vt[2 * b][:, he * 64:(he + 1) * 64],
                                 rhs=e0[:, 0:256], start=False, stop=False)
                nc.tensor.matmul(o_o,
                                 lhsT=vt[2 * b][:, ho * 64:(ho + 1) * 64],
                                 rhs=e0[:, 256:512], start=False, stop=False)
                nc.tensor.matmul(o_e[:, 128:256],
                                 lhsT=vt[2 * b + 1][:, he * 64:(he + 1) * 64],
                                 rhs=e1[:, 0:128], start=False, stop=True)
                nc.tensor.matmul(o_o[:, 128:256],
                                 lhsT=vt[2 * b + 1][:, ho * 64:(ho + 1) * 64],
                                 rhs=e1[:, 128:256], start=False, stop=True)
                linv = at_p.tile([128, 256], F32, tag="linv")
                nc.vector.reciprocal(out=linv[0:64, :], in_=l_e)
                nc.vector.reciprocal(out=linv[64:128, :], in_=l_o)
                if DBG == "linv" and l == 0 and b == 7:
                    nc.sync.dma_start(out=dbg_ap[j], in_=linv)
                nc.vector.tensor_tensor(out=oT[j][0:64, n0:n0 + 256],
                                        in0=o_e, in1=linv[0:64, :], op=OP.mult)
                nc.vector.tensor_tensor(out=oT[j][64:128, n0:n0 + 256],
                                        in0=o_o, in1=linv[64:128, :],
                                        op=OP.mult)

        def proj_tile(t, wproj_s, bproj_s):
            ps = big_ps.tile([128, 512], F32, tag="big")
            for k in range(KT):
                nc.tensor.matmul(ps[:, 0:D],
                                 lhsT=oT[k][:, t * 128:(t + 1) * 128],
                                 rhs=wproj_s[:, k, :],
                                 start=(k == 0), stop=(not biases and k == KT - 1))
            if biases:
                nc.tensor.matmul(ps[:, 0:D], lhsT=ones_s, rhs=bproj_s,
                                 start=False, stop=True)
            nc.vector.tensor_tensor(out=x[t], in0=x[t], in1=ps[:, 0:D],
                                    op=OP.add)

        def ff_chunk(n, wff1_s, wff2_s, bff1_s, bff2_s):
            ns = slice(n * 512, (n + 1) * 512)
            hh = [h_p.tile([128, 512], BF16, tag=f"h{f}", name=f"h{f}")
              for f in range(FT)]
            for f in range(FT):
                ps = big_ps.tile([128, 512], F32, tag="big")
                for k in range(KT):
                    nc.tensor.matmul(
                        ps, lhsT=wff1_s[:, k, f * 128:(f + 1) * 128],
                        rhs=xnT[:, k, ns], start=(k == 0), stop=(k == KT - 1))
                if biases:
                    nc.scalar.activation(out=hh[f], in_=ps, func=AF.Relu,
                                         bias=bff1_s[:, f:f + 1])
                else:
                    nc.scalar.activation(out=hh[f], in_=ps, func=AF.Relu)
            for tt in range(4):
                t = n * 4 + tt
                ps = big_ps.tile([128, 512], F32, tag="big")
                for f in range(FT):
                    nc.tensor.matmul(ps[:, 0:D],
                                     lhsT=hh[f][:, tt * 128:(tt + 1) * 128],
                                     rhs=wff2_s[:, f, :], start=(f == 0),
                                     stop=(not biases and f == FT - 1))
                if biases:
                    nc.tensor.matmul(ps[:, 0:D], lhsT=ones_s, rhs=bff2_s,
                                     start=False, stop=True)
                nc.vector.tensor_tensor(out=x[t], in0=x[t], in1=ps[:, 0:D],
                                        op=OP.add)

        for l in range(n_layers):
            # ---- layer weights (double-buffered pool) ----
            wqkv_s = w_p.tile([128, KT, 3 * D], BF16, tag="wqkv")
            nc.sync.dma_start(out=wqkv_s, in_=wqkv[l])
            wproj_s = w_p.tile([128, KT, D], BF16, tag="wproj")
            nc.sync.dma_start(out=wproj_s, in_=wproj[l])
            wff1_s = w_p.tile([128, KT, FF], BF16, tag="wff1")
            nc.sync.dma_start(out=wff1_s, in_=wff1[l])
            wff2_s = w_p.tile([128, FT, D], BF16, tag="wff2")
            nc.sync.dma_start(out=wff2_s, in_=wff2[l])
            bqk_s = bff1_s = bproj_s = bff2_s = None
            if biases:
                bqk_s = w_p.tile([128, 6], F32, tag="bqk")
                nc.sync.dma_start(out=bqk_s, in_=bqk[l])
                bproj_s = w_p.tile([1, D], BF16, tag="bproj")
                nc.sync.dma_start(out=bproj_s, in_=bproj[l])
                bff1_s = w_p.tile([128, FT], F32, tag="bff1")
                nc.sync.dma_start(out=bff1_s, in_=bff1[l])
                bff2_s = w_p.tile([1, D], BF16, tag="bff2")
                nc.sync.dma_start(out=bff2_s, in_=bff2[l])

            # ---- LN1 (first 8 tiles), qk/v chunks 0-1 ----
            for t in range(8):
                ln_tile(t)
            qkv_chunk(0, wqkv_s, bqk_s)
            for t in range(8, TT):
                ln_tile(t)
            qkv_chunk(1, wqkv_s, bqk_s)
            if l == 0:
                dump3("xnT", xnT)

            def post_attn(b):
                for t in (2 * b, 2 * b + 1):
                    proj_tile(t, wproj_s, bproj_s)
                    ln2_tile(t)

            def ln2_tile(t):
                ln_tile(t)

            attn_batch(0, l)
            attn_batch(1, l)
            if l == 0:
                dump("vt", vt)
                dump("qT", qT)
                dump("kT", kT)
            qkv_chunk(2, wqkv_s, bqk_s)
            post_attn(0)
            post_attn(1)
            attn_batch(2, l)
            attn_batch(3, l)
            qkv_chunk(3, wqkv_s, bqk_s)
            post_attn(2)
            post_attn(3)
            ff_chunk(0, wff1_s, wff2_s, bff1_s, bff2_s)
            attn_batch(4, l)
            attn_batch(5, l)
            post_attn(4)
            post_attn(5)
            ff_chunk(1, wff1_s, wff2_s, bff1_s, bff2_s)
            attn_batch(6, l)
            attn_batch(7, l)
            if l == 0:
                dump("oT", oT)
            post_attn(6)
            post_attn(7)
            if l == 0:
                dump("x1", x)
            ff_chunk(2, wff1_s, wff2_s, bff1_s, bff2_s)
            ff_chunk(3, wff1_s, wff2_s, bff1_s, bff2_s)
            if l == 0:
                dump("x2", x)

        # ---- final LN + head ----
        for t in range(TT):
            ln_tile(t)
            ps = big_ps.tile([128, 512], F32, tag="big")
            for k in range(KT):
                nc.tensor.matmul(ps[:, 0:V],
                                 lhsT=xnT[:, k, t * 128:(t + 1) * 128],
                                 rhs=whead_s[:, k, :],
                                 start=(k == 0), stop=(not biases and k == KT - 1))
            if biases:
                nc.tensor.matmul(ps[:, 0:V], lhsT=ones_s, rhs=bhead_s,
                                 start=False, stop=True)
            lt = cp_p.tile([128, V], F32, tag="logit")
            nc.scalar.activation(out=lt, in_=ps[:, 0:V], func=AF.Identity)
            nc.sync.dma_start(out=logits[t * 128:(t + 1) * 128, :], in_=lt)

    _split_multi_waits(nc)
    return nc


def prepare_host_inputs(idx, tok_emb, pos_emb, ln1_w, ln1_b, wq, wk, wv,
                        proj_w, proj_b, ln2_w, ln2_b, ff_w1, ff_b1, ff_w2,
                        ff_b2, lnf_w, lnf_b, head_w, head_b, n_layers=L,
                        biases=False):
    f32 = np.float32
    bf = ml_dtypes.bfloat16

    def kt_tiles(w, ncols):  # [D, ncols] -> [128, KT, ncols]
        return np.ascontiguousarray(
            w.reshape(-1, 128, ncols).transpose(1, 0, 2))

    wqkv_l, wproj_l, wff1_l, wff2_l = [], [], [], []
    bqk_l, bproj_l, bff1_l, bff2_l = [], [], [], []
    for l in range(n_layers):
        q2 = np.asarray(wq[l]).transpose(1, 0, 2).reshape(D, D).astype(f32)
        k2 = np.asarray(wk[l]).transpose(1, 0, 2).reshape(D, D).astype(f32)
        v2 = np.asarray(wv[l]).transpose(1, 0, 2).reshape(D, D).astype(f32)
        l1w = np.asarray(ln1_w[l], f32)
        l1b = np.asarray(ln1_b[l], f32)
        qf = l1w[:, None] * q2
        kf = l1w[:, None] * k2
        vf = l1w[:, None] * v2
        wqkv_l.append(kt_tiles(np.concatenate([qf, kf, vf], axis=1), 3 * D))
        wproj_l.append(kt_tiles(np.asarray(proj_w[l], f32), D))
        w1f = np.asarray(ln2_w[l], f32)[:, None] * np.asarray(ff_w1[l], f32)
        wff1_l.append(kt_tiles(w1f, FF))
        wff2_l.append(np.ascontiguousarray(
            np.asarray(ff_w2[l], f32).reshape(FT, 128, D).transpose(1, 0, 2)))
        if biases:
            bq = l1b @ q2
            bk = l1b @ k2
            bvv = l1b @ v2
            bqk_l.append(np.concatenate(
                [bq.reshape(KT, 128).T, bk.reshape(KT, 128).T], axis=1))
            # v bias folded through softmax into proj bias (rows sum to 1)
            bproj_l.append(
                (np.asarray(proj_b[l], f32) + bvv @ np.asarray(proj_w[l], f32)
                 ).reshape(1, D))
            b1f = np.asarray(ff_b1[l], f32) + np.asarray(ln2_b[l], f32) @ \
                np.asarray(ff_w1[l], f32)
            bff1_l.append(np.ascontiguousarray(b1f.reshape(FT, 128).T))
            bff2_l.append(np.asarray(ff_b2[l], f32).reshape(1, D))

    whf = np.asarray(lnf_w, f32)[:, None] * np.asarray(head_w, f32)

    def stk(lst, shape, dt):
        if lst:
            return np.stack(lst).astype(dt)
        return np.zeros((0,) + shape, dt)

    shared = {
        "triW": (np.triu(np.full((128, 128), NEG, f32), k=1)).astype(bf),
        "identI2": np.concatenate([np.eye(128, dtype=f32)] * 2, axis=1).astype(bf),
        "ones64": np.ones((128, 64), bf),
        "wqkv": stk(wqkv_l, (128, KT, 3 * D), bf),
        "wproj": stk(wproj_l, (128, KT, D), bf),
        "wff1": stk(wff1_l, (128, KT, FF), bf),
        "wff2": stk(wff2_l, (128, FT, D), bf),
        "whead": kt_tiles(whf, V).astype(bf),
    }
    if biases:
        shared.update({
            "bqk": stk(bqk_l, (128, 6), f32),
            "bproj": stk(bproj_l, (1, D), bf),
            "bff1": stk(bff1_l, (128, FT), f32),
            "bff2": stk(bff2_l, (1, D), bf),
            "bhead": (np.asarray(head_b, f32) +
                      np.asarray(lnf_b, f32) @ np.asarray(head_w, f32)
                      ).reshape(1, V).astype(bf),
            "ones_row": np.ones((1, 128), bf),
        })

    idx = np.asarray(idx)
    te = np.asarray(tok_emb, f32)
    pe = np.asarray(pos_emb, f32)[None, :T, :]  # [1, T, D]
    in_maps = []
    for c in range(NCORES):
        ib = idx[c * BPC:(c + 1) * BPC]                      # [BPC, T]
        x0 = te[ib] + pe                                     # [BPC, T, D]
        in_maps.append({**shared,
                        "x0": x0.reshape(TT, 128, D).astype(f32)})
    return in_maps


def _biases_nonzero(inputs):
    for k in ("ln1_b", "ln2_b", "lnf_b", "proj_b", "ff_b1", "ff_b2", "head_b"):
        if np.any(np.asarray(inputs[k])):
            return True
    return False


_CACHED = {}


def kernel(**inputs):
    biases = _biases_nonzero(inputs)
    key = (L, biases)
    if key not in _CACHED:
        _CACHED[key] = build(L, biases=biases)
    nc = _CACHED[key]
    in_maps = prepare_host_inputs(**inputs, n_layers=L, biases=biases)
    res = run_bass_kernel_spmd(nc, in_maps, list(range(NCORES)))
    out = np.concatenate(
        [res.results[c]["logits"].reshape(BPC, T, V) for c in range(NCORES)],
        axis=0)
    return out


# revision 76
# speedup vs baseline: 1.0312x; 1.0312x over previous
"""Trainium2 Bass kernel for a 6-layer GPT-style transformer
(B=64, T=256, V=65, D=384, H=6, FF=1536), data-parallel over batch on 8
NeuronCores (8 batch elements = 2048 tokens per core).

Design notes (restructured from the baseline for engine balance):
  - Embedding (tok_emb[idx] + pos) is computed host-side and DMA'd in as
    bf16; layer 0 reads it directly (LN1 source + proj residual base) so
    no f32 upconvert is needed.
  - Residual x token-major fp32; LayerNorm via bn_stats/bn_aggr + fused
    tensor_scalar, cast bf16 and DMA-transposed into feature-major xnT.
    LN scale/bias folded into downstream weights host-side.
  - qT/kT feature-major (weights-stationary, N=512 streams); v token-major.
  - Attention per (batch, head); for each head everything that writes a
    given PSUM bank shares one PE tile position or is dependency-ordered
    (concurrent row-group-packed matmuls draining into one bank fault on
    hardware):
      * scores in one [128,384] bank (keys 0:128 x q 0:256 | keys 128:256
        x q 128:256); causal masking accumulates -30000 * triangle into
        the diagonal blocks via extra matmuls (lhsT = strictly-upper
        -30000 triangle, rhs = identity) - no element-wise mask op.
      * one Exp per head (bf16 out), softmax denominators via ones-matmul
        row sums into the head's l/o bank, one DVE reciprocal, and
        normalization fused into the single PSUM->SBUF copy of the
        attention output; odd heads accumulate on PSUM partitions 64:127
        so both heads land in oT without any SBUF-SBUF DMA.
  - proj/ff2 token-major (activation-stationary); ff1 feature-major.
  - All per-feature biases are zero for this model instance; the fast
    graph (biases=False) omits them. biases=True adds K=1 ones-row bias
    matmuls and ACT bias operands (general path).
  - Issue order software-pipelines across phases and layers: ff chunks
    are interleaved between attention batches, the next layer's LN1 (or
    the final LN + head) runs inside each layer's ff tail, and the next
    layer's qk/v chunks fill the layer-boundary bubble.
"""

import numpy as np
import ml_dtypes

import concourse.bass as bass
import concourse.mybir as mybir
import concourse.tile as tile
from concourse.bass_utils import run_bass_kernel_spmd
from contextlib import ExitStack

F32 = mybir.dt.float32
BF16 = mybir.dt.bfloat16
AF = mybir.ActivationFunctionType
OP = mybir.AluOpType

B, T, V, D, H, L = 64, 256, 65, 384, 6, 6
HD = D // H          # 64
FF = 4 * D           # 1536
EPS = 1e-5
SCALE = D ** -0.5
NEG = -30000.0

NCORES = 8
BPC = B // NCORES    # 8 batch elements per core
NTOK = BPC * T       # 2048 tokens per core
TT = NTOK // 128     # 16 token tiles
KT = D // 128        # 3 feature tiles
FT = FF // 128       # 12 ff tiles
NCH = 4              # 512-token chunks
HP = H // 2          # 3 head pairs


def _split_multi_waits(nc):
    """This walrus build rejects >1 sync wait per instruction; hoist extras
    onto standalone EventSemaphore instructions on the same engine queue."""
    ctr = 0
    for func in nc.m.functions:
        for bb in func.blocks:
            insts = list(bb.instructions)
            out = []
            changed = False
            for inst in insts:
                si = inst.sync_info
                if si is not None and len(si.on_wait) > 1:
                    waits = list(si.on_wait)
                    for w in waits[:-1]:
                        ev = mybir.InstEventSemaphore(
                            name=f"splitwait_{ctr}", ins=[], outs=[])
                        ctr += 1
                        ev.engine = inst.engine
                        ev.sync_info = mybir.SyncInfo(on_wait=[w], on_update=[])
                        nc.register_instruction(ev, overwrite=True)
                        out.append(ev)
                    inst.sync_info = mybir.SyncInfo(
                        on_wait=[waits[-1]], on_update=list(si.on_update))
                    changed = True
                out.append(inst)
            if changed:
                bb.instructions = out


DBG = None  # stage tag set by tests


def build(n_layers=L, biases=False):
    nc = bass.Bass(trn_type="TRN2", num_devices=NCORES)

    def din(name, shape, dt):
        return nc.dram_tensor(name, shape, dt, kind="ExternalInput").ap()

    x0_in = din("x0", [128, TT, D], BF16)
    # packed constants: cols 0:128 strictly-upper -30000 triangle,
    # 128:384 [I|I], 384:448 ones
    cpack = din("cpack", [128, 448], BF16)
    if n_layers:
        wqkv = din("wqkv", [n_layers, 128, KT, 3 * D], BF16)
        wproj = din("wproj", [n_layers, 128, KT, D], BF16)
        wff1 = din("wff1", [n_layers, 128, KT, FF], BF16)
        wff2 = din("wff2", [n_layers, 128, FT, D], BF16)
        if biases:
            bqk = din("bqk", [n_layers, 128, 6], F32)
            bproj = din("bproj", [n_layers, 1, D], BF16)
            bff1 = din("bff1", [n_layers, 128, FT], F32)
            bff2 = din("bff2", [n_layers, 1, D], BF16)
    whead = din("whead", [128, KT, V], BF16)
    if biases:
        bhead = din("bhead", [1, V], BF16)
        ones_row = din("ones_row", [1, 128], BF16)
    logits = nc.dram_tensor("logits", [NTOK, V], F32, kind="ExternalOutput").ap()

    dbg_spec = {
        "x0": (TT, 128, D, F32), "xnT": (KT, 128, NTOK, BF16),
        "qT": (KT, 128, NTOK, BF16), "kT": (KT, 128, NTOK, BF16),
        "vt": (TT, 128, D, BF16), "oT": (KT, 128, NTOK, BF16),
        "x1": (TT, 128, D, F32), "x2": (TT, 128, D, F32),
        "e0": (HP, 128, 512, BF16), "linv": (HP, 128, 256, F32),
    }
    dbg_ap = None
    if DBG is not None:
        n, p, c, dt = dbg_spec[DBG]
        dbg_ap = nc.dram_tensor("dbg", [n, p, c], dt, kind="ExternalOutput").ap()

    def dump(tag, tiles):
        if DBG == tag:
            for i, tl in enumerate(tiles):
                nc.sync.dma_start(out=dbg_ap[i], in_=tl)

    def dump3(tag, t3):
        if DBG == tag:
            for i in range(KT):
                nc.sync.dma_start(out=dbg_ap[i], in_=t3[:, i, :])

    with tile.TileContext(nc) as tc, ExitStack() as ctx:
        pool = lambda name, bufs: ctx.enter_context(tc.tile_pool(name=name, bufs=bufs))
        const_p = pool("const", 1)
        xres_p = pool("xres", 1)
        xnT_p = pool("xnT", 1)
        qkT_p = pool("qkT", 1)
        v_p = pool("vtok", 1)
        oT_p = pool("oT", 1)
        h_p = pool("hff", 2)
        w_p = pool("wts", 2)
        ln_p = pool("ln", 6)
        at_p = pool("attn", 6)
        e_p = pool("epool", 14)
        cp_p = pool("cpy", 3)
        big_ps = ctx.enter_context(
            tc.tile_pool(name="big_ps", bufs=3, space="PSUM"))
        sc_ps = ctx.enter_context(
            tc.tile_pool(name="sc_ps", bufs=2, space="PSUM"))
        lo_ps = ctx.enter_context(
            tc.tile_pool(name="lo_ps", bufs=3, space="PSUM"))

        # ---- constants (one packed DMA) ----
        cpack_s = const_p.tile([128, 448], BF16)
        nc.sync.dma_start(out=cpack_s, in_=cpack)
        tri_s = cpack_s[:, 0:128]
        id2_s = cpack_s[:, 128:384]
        ones64_s = cpack_s[:, 384:448]
        eps_t = const_p.tile([128, 1], F32)
        nc.vector.memset(eps_t, EPS)
        if biases:
            bhead_s = const_p.tile([1, V], BF16)
            nc.sync.dma_start(out=bhead_s, in_=bhead)
            ones_s = const_p.tile([1, 128], BF16)
            nc.sync.dma_start(out=ones_s, in_=ones_row)

        # ---- persistent activation tiles ----
        xall = xres_p.tile([128, TT, D], F32, tag="xall", name="xall")
        x = [xall[:, t, :] for t in range(TT)]
        x0bf = xres_p.tile([128, TT, D], BF16, tag="x0bf", name="x0bf")
        x0s = [x0bf[:, t, :] for t in range(TT)]
        xnT = xnT_p.tile([128, KT, NTOK], BF16, tag="xnT", name="xnT")
        qT = [qkT_p.tile([128, NTOK], BF16, tag=f"qT{k}", name=f"qT{k}") for k in range(KT)]
        kT = [qkT_p.tile([128, NTOK], BF16, tag=f"kT{k}", name=f"kT{k}") for k in range(KT)]
        vt = [v_p.tile([128, D], BF16, tag=f"v{t}", name=f"v{t}") for t in range(TT)]
        oT = [oT_p.tile([128, NTOK], BF16, tag=f"oT{k}", name=f"oT{k}") for k in range(KT)]

        # ---- load x0 (bf16, 4 grouped DMAs so LN can start early).
        # Layer 0 reads x0bf directly (LN1 + proj residual base), so no
        # f32 upconvert is needed; xall is first written by layer 0's proj.
        whead_s = const_p.tile([128, KT, V], BF16)
        for g in range(2):
            nc.sync.dma_start(out=x0bf[:, 4 * g:4 * (g + 1), :],
                              in_=x0_in[:, 4 * g:4 * (g + 1), :])
        dump("x0", x0s)

        def ln_tile(t, src=None):
            """token-major LN of src (default x[t]) -> bf16 -> DMA-transpose
            into xnT."""
            if src is None:
                src = x[t]
            stats = ln_p.tile([128, 6], F32, tag="stats")
            nc.vector.bn_stats(out=stats, in_=src)
            mv = ln_p.tile([128, 2], F32, tag="mv")
            nc.vector.bn_aggr(out=mv, in_=stats)
            rstd = ln_p.tile([128, 1], F32, tag="rstd")
            nc.scalar.activation(out=rstd, in_=mv[:, 1:2], func=AF.Sqrt,
                                 bias=eps_t)
            nc.vector.reciprocal(out=rstd, in_=rstd)
            xn16 = ln_p.tile([128, D], BF16, tag="xn16")
            nc.vector.tensor_scalar(out=xn16, in0=src, scalar1=mv[:, 0:1],
                                    scalar2=rstd, op0=OP.subtract, op1=OP.mult)
            nc.sync.dma_start(out=xnT[:, :, t * 128:(t + 1) * 128],
                              in_=xn16, transpose=True)

        def qkv_chunk(n, wqkv_s, bqk_s):
            """qT/kT (feature-major) and v (token-major) for 512-token chunk."""
            ns = slice(n * 512, (n + 1) * 512)
            for m in range(6):
                dst = qT[m] if m < KT else kT[m - KT]
                ps = big_ps.tile([128, 512], F32, tag="big")
                for k in range(KT):
                    nc.tensor.matmul(
                        ps, lhsT=wqkv_s[:, k, m * 128:(m + 1) * 128],
                        rhs=xnT[:, k, ns], start=(k == 0), stop=(k == KT - 1))
                if biases:
                    nc.scalar.activation(out=dst[:, ns], in_=ps, func=AF.Identity,
                                         bias=bqk_s[:, m:m + 1])
                else:
                    nc.scalar.activation(out=dst[:, ns], in_=ps, func=AF.Identity)
            for tt in range(4):
                t = n * 4 + tt
                ps = big_ps.tile([128, 512], F32, tag="big")
                for k in range(KT):
                    nc.tensor.matmul(ps[:, 0:D],
                                     lhsT=xnT[:, k, t * 128:(t + 1) * 128],
                                     rhs=wqkv_s[:, k, 2 * D:3 * D],
                                     start=(k == 0), stop=(k == KT - 1))
                nc.scalar.activation(out=vt[t], in_=ps[:, 0:D], func=AF.Identity)

        def attn_scores(b, l):
            """score + mask matmuls and exps for all heads of batch b.

            All matmuls that write a given PSUM bank share one PE tile
            position (or are dependency-ordered), since concurrent
            row-group-packed matmuls draining into the same bank fault on
            hardware."""
            n0 = b * T
            es = []
            for h in range(H):
                j, r = h // 2, (h % 2) * 64
                # one bank per head: cols 0:256 = keys 0:128 x q 0:256,
                # cols 256:384 = keys 128:256 x q 128:256
                sc = sc_ps.tile([128, 384], F32, tag="sc")
                nc.tensor.matmul(sc[:, 0:256],
                                 lhsT=kT[j][r:r + 64, n0:n0 + 128],
                                 rhs=qT[j][r:r + 64, n0:n0 + 256],
                                 start=True, stop=False)
                nc.tensor.matmul(sc[:, 256:384],
                                 lhsT=kT[j][r:r + 64, n0 + 128:n0 + 256],
                                 rhs=qT[j][r:r + 64, n0 + 128:n0 + 256],
                                 start=False, stop=False)
                nc.tensor.matmul(sc[:, 0:128], lhsT=tri_s,
                                 rhs=id2_s[:, 0:128], start=False, stop=False)
                nc.tensor.matmul(sc[:, 256:384], lhsT=tri_s,
                                 rhs=id2_s[:, 0:128], start=False, stop=True)
                e = e_p.tile([128, 384], BF16, tag="e")
                nc.scalar.activation(out=e, in_=sc, func=AF.Exp, scale=SCALE)
                es.append(e)
            if DBG == "e0" and l == 0 and b == 7:
                for j in range(HP):
                    nc.sync.dma_start(out=dbg_ap[j][:, 0:384], in_=es[2 * j])
            return es

        def attn_lo(b, es, l):
            """softmax denominators, attention output and normalization."""
            n0 = b * T
            for h in range(H):
                j, r = h // 2, (h % 2) * 64
                e = es[h]
                # per-head l/o bank on the head's partition-row range
                lop = lo_ps.tile([128, 512], F32, tag="lo")
                l_ps = lop[r:r + 64, 0:256]
                o_ps = lop[r:r + 64, 256:512]
                nc.tensor.matmul(l_ps, lhsT=ones64_s,
                                 rhs=e[:, 0:256], start=True, stop=False)
                nc.tensor.matmul(l_ps[:, 128:256], lhsT=ones64_s,
                                 rhs=e[:, 256:384], start=False, stop=False)
                nc.tensor.matmul(o_ps,
                                 lhsT=vt[2 * b][:, h * 64:(h + 1) * 64],
                                 rhs=e[:, 0:256], start=False, stop=False)
                nc.tensor.matmul(o_ps[:, 128:256],
                                 lhsT=vt[2 * b + 1][:, h * 64:(h + 1) * 64],
                                 rhs=e[:, 256:384], start=False, stop=True)
                linv = at_p.tile([128, 256], F32, tag="linv")
                nc.vector.reciprocal(out=linv[r:r + 64, :], in_=l_ps)
                if DBG == "linv" and l == 0 and b == 7 and r == 0:
                    nc.sync.dma_start(out=dbg_ap[j], in_=linv)
                nc.vector.tensor_tensor(out=oT[j][r:r + 64, n0:n0 + 256],
                                        in0=o_ps, in1=linv[r:r + 64, :],
                                        op=OP.mult)

        def attn_batch(b, l):
            attn_lo(b, attn_scores(b, l), l)

        def proj_tile(t, wproj_s, bproj_s, xin=None):
            if xin is None:
                xin = x[t]
            ps = big_ps.tile([128, 512], F32, tag="big")
            for k in range(KT):
                nc.tensor.matmul(ps[:, 0:D],
                                 lhsT=oT[k][:, t * 128:(t + 1) * 128],
                                 rhs=wproj_s[:, k, :],
                                 start=(k == 0), stop=(not biases and k == KT - 1))
            if biases:
                nc.tensor.matmul(ps[:, 0:D], lhsT=ones_s, rhs=bproj_s,
                                 start=False, stop=True)
            nc.vector.tensor_tensor(out=x[t], in0=xin, in1=ps[:, 0:D],
                                    op=OP.add)

        def ff_chunk(n, wff1_s, wff2_s, bff1_s, bff2_s):
            ns = slice(n * 512, (n + 1) * 512)
            hh = [h_p.tile([128, 512], BF16, tag=f"h{f}", name=f"h{f}")
              for f in range(FT)]
            for f in range(FT):
                ps = big_ps.tile([128, 512], F32, tag="big")
                for k in range(KT):
                    nc.tensor.matmul(
                        ps, lhsT=wff1_s[:, k, f * 128:(f + 1) * 128],
                        rhs=xnT[:, k, ns], start=(k == 0), stop=(k == KT - 1))
                if biases:
                    nc.scalar.activation(out=hh[f], in_=ps, func=AF.Relu,
                                         bias=bff1_s[:, f:f + 1])
                else:
                    nc.scalar.activation(out=hh[f], in_=ps, func=AF.Relu)
            for tt in range(4):
                t = n * 4 + tt
                ps = big_ps.tile([128, 512], F32, tag="big")
                for f in range(FT):
                    nc.tensor.matmul(ps[:, 0:D],
                                     lhsT=hh[f][:, tt * 128:(tt + 1) * 128],
                                     rhs=wff2_s[:, f, :], start=(f == 0),
                                     stop=(not biases and f == FT - 1))
                if biases:
                    nc.tensor.matmul(ps[:, 0:D], lhsT=ones_s, rhs=bff2_s,
                                     start=False, stop=True)
                nc.vector.tensor_tensor(out=x[t], in0=x[t], in1=ps[:, 0:D],
                                        op=OP.add)

        def head_tile(t, src=None):
            """final LN + head matmul + logits DMA for one token tile."""
            ln_tile(t, src=src)
            ps = big_ps.tile([128, 512], F32, tag="big")
            for k in range(KT):
                nc.tensor.matmul(ps[:, 0:V],
                                 lhsT=xnT[:, k, t * 128:(t + 1) * 128],
                                 rhs=whead_s[:, k, :],
                                 start=(k == 0), stop=(not biases and k == KT - 1))
            if biases:
                nc.tensor.matmul(ps[:, 0:V], lhsT=ones_s, rhs=bhead_s,
                                 start=False, stop=True)
            lt = cp_p.tile([128, V], F32, tag="logit")
            nc.scalar.activation(out=lt, in_=ps[:, 0:V], func=AF.Identity)
            nc.gpsimd.dma_start(out=logits[t * 128:(t + 1) * 128, :], in_=lt)

        def load_wqkv(l):
            wqkv_s = w_p.tile([128, KT, 3 * D], BF16, tag="wqkv")
            nc.sync.dma_start(out=wqkv_s, in_=wqkv[l])
            bqk_s = None
            if biases:
                bqk_s = w_p.tile([128, 6], F32, tag="bqk")
                nc.sync.dma_start(out=bqk_s, in_=bqk[l])
            return wqkv_s, bqk_s

        def load_wrest(l):
            wproj_s = w_p.tile([128, KT, D], BF16, tag="wproj")
            nc.sync.dma_start(out=wproj_s, in_=wproj[l])
            wff1_s = w_p.tile([128, KT, FF], BF16, tag="wff1")
            nc.sync.dma_start(out=wff1_s, in_=wff1[l])
            wff2_s = w_p.tile([128, FT, D], BF16, tag="wff2")
            nc.sync.dma_start(out=wff2_s, in_=wff2[l])
            b = [None] * 3
            if biases:
                bproj_s = w_p.tile([1, D], BF16, tag="bproj")
                nc.sync.dma_start(out=bproj_s, in_=bproj[l])
                bff1_s = w_p.tile([128, FT], F32, tag="bff1")
                nc.sync.dma_start(out=bff1_s, in_=bff1[l])
                bff2_s = w_p.tile([1, D], BF16, tag="bff2")
                nc.sync.dma_start(out=bff2_s, in_=bff2[l])
                b = [bproj_s, bff1_s, bff2_s]
            return (wproj_s, wff1_s, wff2_s) + tuple(b)

        for l in range(n_layers):
            if l:
                wqkv_s, bqk_s = Wq
            last = l == n_layers - 1

            def tail_ln(ts):
                # produce next layer's LN1 (or the final LN + head) as soon
                # as each ff chunk finishes, keeping it off the layer
                # critical path
                for t in ts:
                    if last:
                        head_tile(t)
                    else:
                        ln_tile(t)

            if l == 0:
                # prologue: LN1 of layer 0 (from bf16 x0) interleaved with
                # qk/v chunks; remaining input DMAs deferred so the first
                # transposes get SP/HWDGE slots immediately
                for t in range(4):
                    ln_tile(t, src=x0s[t])
                wqkv_s, bqk_s = load_wqkv(0)
                for g in range(2, 4):
                    nc.sync.dma_start(out=x0bf[:, 4 * g:4 * (g + 1), :],
                                      in_=x0_in[:, 4 * g:4 * (g + 1), :])
                nc.sync.dma_start(out=whead_s, in_=whead)
                for t in range(4, 8):
                    ln_tile(t, src=x0s[t])
                Wr = load_wrest(0)
                qkv_chunk(0, wqkv_s, bqk_s)
                for t in range(8, TT):
                    ln_tile(t, src=x0s[t])
                qkv_chunk(1, wqkv_s, bqk_s)
            # for l > 0, qkv chunks 0-1 were issued in the previous layer's
            # tail
            wproj_s, wff1_s, wff2_s, bproj_s, bff1_s, bff2_s = Wr
            if l == 0:
                dump3("xnT", xnT)
            if not last:
                Wq_next = load_wqkv(l + 1)
                Wr_next = load_wrest(l + 1)

            def post_attn(b):
                for t in (2 * b, 2 * b + 1):
                    proj_tile(t, wproj_s, bproj_s,
                              xin=x0s[t] if l == 0 else None)
                    ln_tile(t)

            attn_batch(0, l)
            if l:
                qkv_chunk(1, wqkv_s, bqk_s)
            attn_batch(1, l)
            if l == 0:
                dump("vt", vt)
                dump("qT", qT)
                dump("kT", kT)
            e2 = attn_scores(2, l)
            e3 = attn_scores(3, l)
            qkv_chunk(2, wqkv_s, bqk_s)
            attn_lo(2, e2, l)
            attn_lo(3, e3, l)
            post_attn(0)
            post_attn(1)
            qkv_chunk(3, wqkv_s, bqk_s)
            post_attn(2)
            post_attn(3)
            ff_chunk(0, wff1_s, wff2_s, bff1_s, bff2_s)
            tail_ln(range(0, 4))
            e4 = attn_scores(4, l)
            e5 = attn_scores(5, l)
            ff_chunk(1, wff1_s, wff2_s, bff1_s, bff2_s)
            attn_lo(4, e4, l)
            attn_lo(5, e5, l)
            post_attn(4)
            post_attn(5)
            tail_ln(range(4, 8))
            e6 = attn_scores(6, l)
            e7 = attn_scores(7, l)
            ff_chunk(2, wff1_s, wff2_s, bff1_s, bff2_s)
            attn_lo(6, e6, l)
            attn_lo(7, e7, l)
            if l == 0:
                dump("oT", oT)
            post_attn(6)
            post_attn(7)
            if l == 0:
                dump("x1", x)
            if not last:
                # fill the layer tail with next layer's first qk/v chunk
                # (xnT regions 0:512 already hold next-layer LN1 data)
                qkv_chunk(0, Wq_next[0], Wq_next[1])
            tail_ln(range(8, 12))
            ff_chunk(3, wff1_s, wff2_s, bff1_s, bff2_s)
            tail_ln(range(12, TT))
            if l == 0:
                dump("x2", x)
            if not last:
                Wq, Wr = Wq_next, Wr_next

        if n_layers == 0:
            for g in range(2, 4):
                nc.sync.dma_start(out=x0bf[:, 4 * g:4 * (g + 1), :],
                                  in_=x0_in[:, 4 * g:4 * (g + 1), :])
            nc.sync.dma_start(out=whead_s, in_=whead)
            for t in range(TT):
                head_tile(t, src=x0s[t])

    _split_multi_waits(nc)
    return nc


def prepare_host_inputs(idx, tok_emb, pos_emb, ln1_w, ln1_b, wq, wk, wv,
                        proj_w, proj_b, ln2_w, ln2_b, ff_w1, ff_b1, ff_w2,
                        ff_b2, lnf_w, lnf_b, head_w, head_b, n_layers=L,
                        biases=False):
    f32 = np.float32
    bf = ml_dtypes.bfloat16

    def kt_tiles(w, ncols):  # [D, ncols] -> [128, KT, ncols]
        return np.ascontiguousarray(
            w.reshape(-1, 128, ncols).transpose(1, 0, 2))

    wqkv_l, wproj_l, wff1_l, wff2_l = [], [], [], []
    bqk_l, bproj_l, bff1_l, bff2_l = [], [], [], []
    for l in range(n_layers):
        q2 = np.asarray(wq[l]).transpose(1, 0, 2).reshape(D, D).astype(f32)
        k2 = np.asarray(wk[l]).transpose(1, 0, 2).reshape(D, D).astype(f32)
        v2 = np.asarray(wv[l]).transpose(1, 0, 2).reshape(D, D).astype(f32)
        l1w = np.asarray(ln1_w[l], f32)
        l1b = np.asarray(ln1_b[l], f32)
        qf = l1w[:, None] * q2
        kf = l1w[:, None] * k2
        vf = l1w[:, None] * v2
        wqkv_l.append(kt_tiles(np.concatenate([qf, kf, vf], axis=1), 3 * D))
        wproj_l.append(kt_tiles(np.asarray(proj_w[l], f32), D))
        w1f = np.asarray(ln2_w[l], f32)[:, None] * np.asarray(ff_w1[l], f32)
        wff1_l.append(kt_tiles(w1f, FF))
        wff2_l.append(np.ascontiguousarray(
            np.asarray(ff_w2[l], f32).reshape(FT, 128, D).transpose(1, 0, 2)))
        if biases:
            bq = l1b @ q2
            bk = l1b @ k2
            bvv = l1b @ v2
            bqk_l.append(np.concatenate(
                [bq.reshape(KT, 128).T, bk.reshape(KT, 128).T], axis=1))
            # v bias folded through softmax into proj bias (rows sum to 1)
            bproj_l.append(
                (np.asarray(proj_b[l], f32) + bvv @ np.asarray(proj_w[l], f32)
                 ).reshape(1, D))
            b1f = np.asarray(ff_b1[l], f32) + np.asarray(ln2_b[l], f32) @ \
                np.asarray(ff_w1[l], f32)
            bff1_l.append(np.ascontiguousarray(b1f.reshape(FT, 128).T))
            bff2_l.append(np.asarray(ff_b2[l], f32).reshape(1, D))

    whf = np.asarray(lnf_w, f32)[:, None] * np.asarray(head_w, f32)

    def stk(lst, shape, dt):
        if lst:
            return np.stack(lst).astype(dt)
        return np.zeros((0,) + shape, dt)

    cpack = np.concatenate(
        [np.triu(np.full((128, 128), NEG, f32), k=1),
         np.eye(128, dtype=f32), np.eye(128, dtype=f32),
         np.ones((128, 64), f32)], axis=1)
    shared = {
        "cpack": cpack.astype(bf),
        "wqkv": stk(wqkv_l, (128, KT, 3 * D), bf),
        "wproj": stk(wproj_l, (128, KT, D), bf),
        "wff1": stk(wff1_l, (128, KT, FF), bf),
        "wff2": stk(wff2_l, (128, FT, D), bf),
        "whead": kt_tiles(whf, V).astype(bf),
    }
    if biases:
        shared.update({
            "bqk": stk(bqk_l, (128, 6), f32),
            "bproj": stk(bproj_l, (1, D), bf),
            "bff1": stk(bff1_l, (128, FT), f32),
            "bff2": stk(bff2_l, (1, D), bf),
            "bhead": (np.asarray(head_b, f32) +
                      np.asarray(lnf_b, f32) @ np.asarray(head_w, f32)
                      ).reshape(1, V).astype(bf),
            "ones_row": np.ones((1, 128), bf),
        })

    idx = np.asarray(idx)
    te = np.asarray(tok_emb, f32)
    pe = np.asarray(pos_emb, f32)[None, :T, :]  # [1, T, D]
    in_maps = []
    for c in range(NCORES):
        ib = idx[c * BPC:(c + 1) * BPC]                      # [BPC, T]
        x0 = te[ib] + pe                                     # [BPC, T, D]
        x0 = np.ascontiguousarray(
            x0.reshape(TT, 128, D).transpose(1, 0, 2)).astype(bf)
        in_maps.append({**shared, "x0": x0})
    return in_maps


def _biases_nonzero(inputs):
    for k in ("ln1_b", "ln2_b", "lnf_b", "proj_b", "ff_b1", "ff_b2", "head_b"):
        if np.any(np.asarray(inputs[k])):
            return True
    return False


_CACHED = {}


def kernel(**inputs):
    biases = _biases_nonzero(inputs)
    key = (L, biases)
    if key not in _CACHED:
        _CACHED[key] = build(L, biases=biases)
    nc = _CACHED[key]
    in_maps = prepare_host_inputs(**inputs, n_layers=L, biases=biases)
    res = run_bass_kernel_spmd(nc, in_maps, list(range(NCORES)))
    out = np.concatenate(
        [res.results[c]["logits"].reshape(BPC, T, V) for c in range(NCORES)],
        axis=0)
    return out
